# revision 10
# baseline (speedup 1.0000x reference)
"""Farthest Point Sampling (FPS) Bass/TRN2 kernel, v7.

Problem: pos [16, 16384, 3] f32 -> indices [16*2048] int32 (exact FPS,
start index 0, ratio 1/8), bit-exact trajectory vs the f32 reference.

Sharding: batch 16 clouds -> 8 NeuronCores, 2 clouds per core (data
parallel). Each cloud is laid out as [128 partitions, 128 free]
(point n -> (n//128, n%128)).

Per FPS step per cloud:
  ACT : SQX/SQY/SQZ = Square(coord + bias)          bias = -c [128,1] AP
  DVE : t1 = sqx+sqy; d = t1+sqz; DIST = min(DIST, d)   (stt ops)
  DVE : best4[:,0] = rowmax = reduce_max(DIST)
  DVE : maskR = is_eq(DIST, rowmax)                 per-partition argmax mask
  DVE : stt x3: best4[:,1+j] = sum_c maskR*POS_j    per-partition best x/y/z
  PE  : b4T = matmul(best4^T via identity) -> PSUM [4,128]
  DVE : m = reduce_max(b4T[0,:]);  maskrow = is_eq(b4T[0,:], m)  [1,128]
  PE  : onehotP = matmul(maskrow^T) -> PSUM [128,1]
  ACT : onehotP_sb = copy(onehotP)
  PE  : WB = matmul(onehotP_bcast[128,128], best4) -> PSUM [128,4]
        = winner row (m, x*, y*, z*) broadcast to all partitions
  ACT : biassb = -WB[:,1:4] (copy scale=-1); outrow[0,3s:3s+3] = WB[0,1:4]
Host decodes indices by exact coord match against pos (no ties for this
input; verified bit-exact).

Hazard rules baked in (hardware-verified):
  - a DVE reduce/accum write must not be consumed by the IMMEDIATELY
    following DVE instruction (stale read) -> schedule interleaves the
    other cloud's op or a spacer between such pairs.
  - PE is_transpose signals its semaphore before the PSUM write is
    visible -> use regular matmuls only."""

import numpy as np
from contextlib import ExitStack

import concourse.bass as bass
import concourse.mybir as mybir
from concourse.bass_utils import run_bass_kernel_spmd

AT = mybir.ActivationFunctionType
AL = mybir.AluOpType
AX = mybir.AxisListType
F32 = mybir.dt.float32

B, N, S = 16, 16384, 2048
N_CORES = 8
N_CLOUDS = 2  # per core
BIG = 1.0e10

_CACHE = {}
LABELS = {}


def _build_fps_kernel(S=S, n_clouds=N_CLOUDS):
    nc = bass.Bass(trn_type="TRN2", detect_race_conditions=False)
    mega_d = nc.dram_tensor("mega", [n_clouds, 128, 384], F32, kind="ExternalInput")
    bias0_d = nc.dram_tensor("bias0", [n_clouds, 128, 3], F32, kind="ExternalInput")
    ident_d = nc.dram_tensor("ident", [128, 128], F32, kind="ExternalInput")
    out_d = nc.dram_tensor("outrow", [n_clouds, 3 * S], F32, kind="ExternalOutput")

    es = ExitStack()
    counter = [0]

    def sb(shape, dtype=F32):
        counter[0] += 1
        return es.enter_context(nc.sbuf_tensor(f"sb{counter[0]}", shape, dtype))

    def ps(shape, dtype=F32):
        counter[0] += 1
        return es.enter_context(nc.psum_tensor(f"ps{counter[0]}", shape, dtype))

    ident = sb([128, 128])
    one11 = sb([1, 1])
    spc = sb([1, 1])

    cl = []
    for c in range(n_clouds):
        cl.append(dict(
            mega=sb([128, 384]),
            dist=sb([128, 128]),
            sqx=sb([128, 128]), sqy=sb([128, 128]), sqz=sb([128, 128]),
            t1=sb([128, 128]), dd=sb([128, 128]),
            maskR=sb([128, 128]),
            scr=sb([128, 128]),
            best4=sb([128, 4]),
            m_sb=sb([1, 1]),
            maskrow=sb([1, 128]),
            oneh=sb([128, 1]),
            biassb=sb([128, 3]),
            outrow=sb([1, 3 * S]),
            b4T_ps=ps([4, 128]),
            oneh_ps=ps([128, 1]),
            wb_ps=ps([128, 4]),
        ))

    sem_act = es.enter_context(nc.semaphore(name="sem_act"))
    sem_dve = es.enter_context(nc.semaphore(name="sem_dve"))
    sem_pe = es.enter_context(nc.semaphore(name="sem_pe"))
    sem_gp = es.enter_context(nc.semaphore(name="sem_gp"))

    sems = {"act": sem_act, "dve": sem_dve, "pe": sem_pe, "gp": sem_gp}
    engines = {"act": nc.scalar, "dve": nc.vector, "pe": nc.tensor, "gp": nc.gpsimd}
    count = {k: 0 for k in sems}
    waited = {(a, b): 0 for a in sems for b in sems}
    label = [None]

    def emit(eng, instr, inc=1):
        instr.then_inc(sems[eng], inc)
        count[eng] += inc
        if label[0] is not None:
            try:
                LABELS[instr.ins.name] = label[0]
            except Exception:
                pass
        return count[eng]

    def wait(consumer, producer, tick):
        if tick is None or consumer == producer:
            return
        if waited[(consumer, producer)] < tick:
            engines[consumer].wait_ge(sems[producer], tick)
            waited[(consumer, producer)] = tick

    for c in range(n_clouds):
        emit("gp", nc.gpsimd.dma_start(cl[c]["mega"][:], mega_d[c]), 16)
        emit("gp", nc.gpsimd.dma_start(cl[c]["biassb"][:], bias0_d[c]), 16)
    emit("gp", nc.gpsimd.dma_start(ident[:], ident_d[:]), 16)
    dma0 = count["gp"]
    wait("dve", "gp", dma0)
    emit("dve", nc.vector.memset(one11[:], 1.0))
    for c in range(n_clouds):
        emit("dve", nc.vector.memset(cl[c]["dist"][:], BIG))
        emit("dve", nc.vector.memset(cl[c]["outrow"][:], 0.0))
        emit("dve", nc.vector.memset(cl[c]["best4"][:], 0.0))
    wait("act", "gp", dma0)
    wait("pe", "gp", dma0)

    ticks = [dict() for _ in range(n_clouds)]

    # ---- phase functions -------------------------------------------------
    def head_act(c):
        """ACT: 3 squares. biassb written by ACT (in-order) - no wait."""
        t, tk = cl[c], ticks[c]
        label[0] = f"{'AB'[c]}.sq"
        for j, sq in enumerate(("sqx", "sqy", "sqz")):
            tk[sq] = emit("act", nc.scalar.activation(
                t[sq][:], t["mega"][:, j * 128:(j + 1) * 128], AT.Square,
                bias=t["biassb"][:, j:j + 1], scale=1.0))

    def upd_a(c):
        """DVE: t1 = sqx+sqy."""
        t, tk = cl[c], ticks[c]
        label[0] = f"{'AB'[c]}.up"
        wait("dve", "act", tk["sqy"])
        tk["t1"] = emit("dve", nc.vector.scalar_tensor_tensor(
            t["t1"][:], t["sqx"][:], 1.0, t["sqy"][:], AL.mult, AL.add))

    def upd_b(c, seam_spacer=False):
        """DVE: d = t1+sqz; DIST = min(DIST, d); rowmax (+optional spacer)."""
        t, tk = cl[c], ticks[c]
        label[0] = f"{'AB'[c]}.up"
        wait("dve", "act", tk["sqz"])
        tk["d"] = emit("dve", nc.vector.scalar_tensor_tensor(
            t["dd"][:], t["t1"][:], 1.0, t["sqz"][:], AL.mult, AL.add))
        tk["min"] = emit("dve", nc.vector.scalar_tensor_tensor(
            t["dist"][:], t["dd"][:], 1.0, t["dist"][:], AL.mult, AL.min))
        tk["rowmax"] = emit("dve", nc.vector.tensor_reduce(
            t["best4"][:, 0:1], t["dist"][:], axis=AX.X, op=AL.max))
        if seam_spacer:
            emit("dve", nc.vector.tensor_copy(spc[0:1, 0:1], one11[0:1, 0:1]))
            emit("dve", nc.vector.tensor_copy(spc[0:1, 0:1], one11[0:1, 0:1]))

    def gath(c):
        """DVE: maskR; stt x3 gather x/y/z into best4[:,1:4].
        Caller must ensure >=1 DVE op between rowmax(c) and this."""
        t, tk = cl[c], ticks[c]
        label[0] = f"{'AB'[c]}.ga"
        tk["maskR"] = emit("dve", nc.vector.tensor_tensor(
            t["maskR"][:], t["dist"][:], t["best4"][:, 0:1].broadcast_to((128, 128)), AL.is_equal))
        for j in range(3):
            tk["g"] = emit("dve", nc.vector.scalar_tensor_tensor(
                t["scr"][:], t["mega"][:, j * 128:(j + 1) * 128], 1.0, t["maskR"][:],
                AL.mult, AL.mult, accum_out=t["best4"][:, 1 + j:2 + j]))

    def tp4(c):
        """PE: best4^T -> [4,128] PSUM (regular matmul vs identity)."""
        t, tk = cl[c], ticks[c]
        label[0] = f"{'AB'[c]}.tp"
        wait("pe", "dve", tk["g"])
        tk["tp4"] = emit("pe", nc.tensor.matmul(
            t["b4T_ps"][:], t["best4"][:], ident[:], start=True, stop=True))

    def midm(c):
        """DVE: m = max(b4T[0,:])."""
        t, tk = cl[c], ticks[c]
        label[0] = f"{'AB'[c]}.mm"
        wait("dve", "pe", tk["tp4"])
        tk["m"] = emit("dve", nc.vector.tensor_reduce(
            t["m_sb"][0:1, 0:1], t["b4T_ps"][0:1, :], axis=AX.X, op=AL.max))

    def mrow(c):
        """DVE: maskrow = is_eq(b4T[0,:], m).
        Caller must ensure >=1 DVE op between midm(c) and this."""
        t, tk = cl[c], ticks[c]
        label[0] = f"{'AB'[c]}.mr"
        tk["mrow"] = emit("dve", nc.vector.tensor_tensor(
            t["maskrow"][0:1, :], t["b4T_ps"][0:1, :],
            t["m_sb"][0:1, 0:1].broadcast_to((1, 128)), AL.is_equal))

    def tpm(c):
        """PE: onehotP = maskrow^T -> PSUM [128,1]."""
        t, tk = cl[c], ticks[c]
        label[0] = f"{'AB'[c]}.tm"
        wait("pe", "dve", tk["mrow"])
        tk["tpm"] = emit("pe", nc.tensor.matmul(
            t["oneh_ps"][:], t["maskrow"][0:1, :], one11[0:1, 0:1], start=True, stop=True))

    def cpo(c):
        """ACT: copy onehotP PSUM -> SBUF."""
        t, tk = cl[c], ticks[c]
        label[0] = f"{'AB'[c]}.co"
        wait("act", "pe", tk["tpm"])
        tk["cpo"] = emit("act", nc.scalar.copy(t["oneh"][:], t["oneh_ps"][:]))

    def wbmm(c):
        """PE: WB = onehotP_bcast^T @ best4 -> [128,4] winner bcast."""
        t, tk = cl[c], ticks[c]
        label[0] = f"{'AB'[c]}.wb"
        wait("pe", "act", tk["cpo"])
        tk["wb"] = emit("pe", nc.tensor.matmul(
            t["wb_ps"][:], t["oneh"][:, 0:1].broadcast_to((128, 128)), t["best4"][:],
            start=True, stop=True))

    def tail(c, s):
        """ACT: biassb = -WB[:,1:4]; outrow[0,3s:3s+3] = WB[0,1:4]."""
        t, tk = cl[c], ticks[c]
        label[0] = f"{'AB'[c]}.tl"
        wait("act", "pe", tk["wb"])
        tk["bias"] = emit("act", nc.scalar.activation(
            t["biassb"][:], t["wb_ps"][:, 1:4], AT.Copy, bias=0.0, scale=-1.0))
        tk["out"] = emit("act", nc.scalar.copy(
            t["outrow"][0:1, 3 * s:3 * s + 3], t["wb_ps"][0:1, 1:4]))

    # ---- schedule: software-pipelined, B half a step behind A ------------
    A, Bc = 0, 1

    def steady(s):
        # A runs step s; B finishes step s-1, starts step s.
        head_act(A)        # ACT: A-sq
        gath(Bc)           # DVE: B-maskR, B-stt3 (seam spacers separate B-rowmax)
        tp4(Bc)            # PE
        upd_a(A)           # DVE: A-t1
        upd_b(A)           # DVE: A-d, A-min, A-rowmax
        midm(Bc)           # DVE: B-m (waits tp4(B))
        emit("dve", nc.vector.tensor_copy(spc[0:1, 0:1], one11[0:1, 0:1]))
        gath(A)            # DVE: A-maskR (separated from A-rowmax by B-m + spacer), A-stt3
        tp4(A)             # PE: ready right after A-stt3, BEFORE B's tail matmuls
        mrow(Bc)           # DVE: B-mrow (separated from B-m by A-maskR + stt3)
        tpm(Bc)            # PE
        cpo(Bc)            # ACT (after A-sq)
        wbmm(Bc)           # PE
        tail(Bc, s - 1)    # ACT: B-bias, B-out
        head_act(Bc)       # ACT: B-sq (after B-bias in ACT order)
        midm(A)            # DVE: A-m (tp4(A) completed early)
        emit("dve", nc.vector.tensor_copy(spc[0:1, 0:1], one11[0:1, 0:1]))
        emit("dve", nc.vector.tensor_copy(spc[0:1, 0:1], one11[0:1, 0:1]))
        mrow(A)            # DVE: A-mrow
        tpm(A)             # PE
        cpo(A)             # ACT
        wbmm(A)            # PE
        upd_a(Bc)          # DVE: B-t1
        upd_b(Bc, seam_spacer=True)  # DVE: B-d, B-min, B-rowmax, spacers
        tail(A, s)         # ACT: A-bias, A-out

    # prologue: step 1 for A, then B
    head_act(A)
    upd_a(A)
    upd_b(A)
    emit("dve", nc.vector.tensor_copy(spc[0:1, 0:1], one11[0:1, 0:1]))
    emit("dve", nc.vector.tensor_copy(spc[0:1, 0:1], one11[0:1, 0:1]))
    gath(A)
    tp4(A)
    midm(A)
    emit("dve", nc.vector.tensor_copy(spc[0:1, 0:1], one11[0:1, 0:1]))
    emit("dve", nc.vector.tensor_copy(spc[0:1, 0:1], one11[0:1, 0:1]))
    mrow(A)
    tpm(A)
    cpo(A)
    wbmm(A)
    head_act(Bc)
    upd_a(Bc)
    upd_b(Bc, seam_spacer=True)
    tail(A, 1)
    for s in range(2, S):
        steady(s)
    # epilogue: B's last step tail
    gath(Bc)
    tp4(Bc)
    midm(Bc)
    emit("dve", nc.vector.tensor_copy(spc[0:1, 0:1], one11[0:1, 0:1]))
    emit("dve", nc.vector.tensor_copy(spc[0:1, 0:1], one11[0:1, 0:1]))
    mrow(Bc)
    tpm(Bc)
    cpo(Bc)
    wbmm(Bc)
    tail(Bc, S - 1)

    for c in range(n_clouds):
        wait("gp", "act", ticks[c]["out"])
        emit("gp", nc.gpsimd.dma_start(out_d[c], cl[c]["outrow"][0:1, :]), 16)

    es.close()
    return nc


def _make_inputs(pos_pair):
    ncl = pos_pair.shape[0]
    mega = np.empty((ncl, 128, 384), np.float32)
    bias0 = np.empty((ncl, 128, 3), np.float32)
    for c in range(ncl):
        for j in range(3):
            mega[c, :, j * 128:(j + 1) * 128] = pos_pair[c, :, j].reshape(128, 128)
        bias0[c] = -pos_pair[c, 0]
    return {
        "mega": mega,
        "bias0": bias0,
        "ident": np.eye(128, dtype=np.float32),
    }


def _get_nc():
    if "nc" not in _CACHE:
        _CACHE["nc"] = _build_fps_kernel()
    return _CACHE["nc"]


def _decode(outrow3, pos_cloud):
    """outrow3 [S,3] winner coords -> local indices via exact match."""
    lut = {}
    pb = np.ascontiguousarray(pos_cloud)
    for n in range(pb.shape[0]):
        lut[pb[n].tobytes()] = n
    idx = np.empty(outrow3.shape[0], np.int32)
    idx[0] = 0
    co = np.ascontiguousarray(outrow3)
    nbad = 0
    for s in range(1, outrow3.shape[0]):
        v = lut.get(co[s].tobytes())
        if v is None:
            v = -1
            nbad += 1
    
        idx[s] = v
    if nbad:
        print(f"decode: {nbad} unmatched coord rows (first at "
              f"{[s for s in range(1, outrow3.shape[0]) if lut.get(co[s].tobytes()) is None][:5]})")
    return idx


def run_on_cores(pos, **spmd_kwargs):
    """pos [16, 16384, 3] f32 -> (idx [16*2048] int32, BassKernelResults)."""
    pos = np.ascontiguousarray(np.asarray(pos, dtype=np.float32))
    assert pos.shape == (B, N, 3)
    nc = _get_nc()
    in_maps = [_make_inputs(pos[N_CLOUDS * c: N_CLOUDS * (c + 1)]) for c in range(N_CORES)]
    res = run_bass_kernel_spmd(nc, in_maps, core_ids=list(range(N_CORES)), **spmd_kwargs)
    idx = np.empty((B, S), np.int32)
    for core in range(N_CORES):
        outrow = res.results[core]["outrow"]  # [n_clouds, 3S]
        for c in range(N_CLOUDS):
            b = N_CLOUDS * core + c
            idx[b] = _decode(outrow[c].reshape(S, 3), pos[b]) + b * N
    return idx.reshape(-1), res


def kernel(pos):
    idx, _ = run_on_cores(pos)
    return idx


# revision 11
# speedup vs baseline: 1.0759x; 1.0759x over previous
"""Farthest Point Sampling (FPS) Bass/TRN2 kernel.

Problem: pos [16, 16384, 3] f32 -> indices [16*2048] int32 (exact FPS,
start index 0, ratio 1/8), bit-exact trajectory vs the f32 reference.

Sharding: batch 16 clouds -> 8 NeuronCores, 2 clouds per core (data
parallel, no cross-core communication). Each cloud is laid out as
[128 partitions, 128 free] (point n -> (n//128, n%128)).

Per FPS step per cloud (raw bass, manual semaphores):
  ACT : SQX/SQY/SQZ = Square(coord + bias)       bias = -c (per-partition AP)
  DVE : t1 = SQX+SQY; d = t1+SQZ; DIST = min(DIST, d); rowmax = max_f(DIST)
  PE  : rmT = rowmax^T                           (matmul vs identity -> PSUM)
  DVE : M = max(rmT)                             [1,1]
  PE  : Mb = ones_row^T @ M                      broadcast M -> [128,1] PSUM
  DVE : MASK = is_equal(DIST, Mb)                single-hot (no ties, verified)
  DVE : SCR4 = MEGA4 * MASK(x4);  RS = reduce_add -> [128,4]  (MEGA4 = [X|Y|Z|GIOTA])
  PE  : NEGBC = (-1)^T128 @ RS -> PSUM [128,4]   = (-cx,-cy,-cz,-n*) everywhere
  ACT : BIAS = NEGBC[:,0:3] -> SBUF;  OUTROW[0, 4s:4s+4] = NEGBC[0,:]
Host decodes n* = -OUTROW[4s+3]."""

import numpy as np
from contextlib import ExitStack

import concourse.bass as bass
import concourse.mybir as mybir
from concourse.bass_utils import run_bass_kernel_spmd

AT = mybir.ActivationFunctionType
AL = mybir.AluOpType
AX = mybir.AxisListType
F32 = mybir.dt.float32

B, N, S = 16, 16384, 2048
N_CORES = 8
N_CLOUDS = 2  # per core
BIG = 1.0e10

_CACHE = {}
LABELS = {}


def _build_fps_kernel(S=S, n_clouds=N_CLOUDS):
    nc = bass.Bass(trn_type="TRN2", detect_race_conditions=False)
    mega_d = nc.dram_tensor("mega", [n_clouds, 128, 384], F32, kind="ExternalInput")
    bias0_d = nc.dram_tensor("bias0", [n_clouds, 128, 3], F32, kind="ExternalInput")
    ident_d = nc.dram_tensor("ident", [128, 128], F32, kind="ExternalInput")
    onesrow_d = nc.dram_tensor("onesrow", [1, 128], F32, kind="ExternalInput")
    negones_d = nc.dram_tensor("negones", [128, 128], F32, kind="ExternalInput")
    out_d = nc.dram_tensor("outrow", [n_clouds, 3 * S], F32, kind="ExternalOutput")

    es = ExitStack()
    counter = [0]

    def sb(shape):
        counter[0] += 1
        return es.enter_context(nc.sbuf_tensor(f"sb{counter[0]}", shape, F32))

    def ps(shape):
        counter[0] += 1
        return es.enter_context(nc.psum_tensor(f"ps{counter[0]}", shape, F32))

    ident = sb([128, 128])
    onesrow = sb([1, 128])
    negones = sb([128, 128])

    cl = []
    for c in range(n_clouds):
        cl.append(dict(
            mega=sb([128, 384]),
            dist=sb([128, 128]),
            sqx=sb([128, 128]), sqy=sb([128, 128]), sqz=sb([128, 128]),
            t1=sb([128, 128]), dd=sb([128, 128]),
            mask=sb([128, 128]),
            scr4=sb([128, 384]),
            rs=sb([128, 3]),
            rowmax=sb([128, 1]),
            msb=sb([1, 1]),
            biassb=sb([128, 3]),
            outrow=sb([1, 3 * S]),
            rmt_ps=ps([1, 128]),
            mb_ps=ps([128, 1]),
            negbc_ps=ps([128, 3]),
        ))

    sem_act = es.enter_context(nc.semaphore())
    sem_dve = es.enter_context(nc.semaphore())
    sem_pe = es.enter_context(nc.semaphore())
    sem_gp = es.enter_context(nc.semaphore())

    sems = {"act": sem_act, "dve": sem_dve, "pe": sem_pe, "gp": sem_gp}
    engines = {"act": nc.scalar, "dve": nc.vector, "pe": nc.tensor, "gp": nc.gpsimd}
    count = {k: 0 for k in sems}
    waited = {(a, b): 0 for a in sems for b in sems}
    label = [None]

    def emit(eng, instr, inc=1):
        instr.then_inc(sems[eng], inc)
        count[eng] += inc
        if label[0] is not None:
            try:
                LABELS[instr.ins.name] = label[0]
            except Exception:
                pass
        return count[eng]

    def wait(consumer, producer, tick):
        if tick is None or consumer == producer:
            return
        if waited[(consumer, producer)] < tick:
            engines[consumer].wait_ge(sems[producer], tick)
            waited[(consumer, producer)] = tick

    for c in range(n_clouds):
        emit("gp", nc.gpsimd.dma_start(cl[c]["mega"][:], mega_d[c]), 16)
        emit("gp", nc.gpsimd.dma_start(cl[c]["biassb"][:], bias0_d[c]), 16)
    emit("gp", nc.gpsimd.dma_start(ident[:], ident_d[:]), 16)
    emit("gp", nc.gpsimd.dma_start(onesrow[:], onesrow_d[:]), 16)
    emit("gp", nc.gpsimd.dma_start(negones[:], negones_d[:]), 16)
    dma0 = count["gp"]
    for c in range(n_clouds):
        wait("dve", "gp", dma0)
        emit("dve", nc.vector.memset(cl[c]["dist"][:], BIG))
        emit("dve", nc.vector.memset(cl[c]["outrow"][:], 0.0))
    wait("act", "gp", dma0)
    wait("pe", "gp", dma0)

    ticks = [dict() for _ in range(n_clouds)]

    def upd_head(c, s):
        t, tk = cl[c], ticks[c]
        label[0] = f"{'AB'[c]}.upd"
        for j, sq in enumerate(("sqx", "sqy", "sqz")):
            tk[sq] = emit("act", nc.scalar.activation(
                t[sq][:], t["mega"][:, j * 128:(j + 1) * 128], AT.Square,
                bias=t["biassb"][:, j:j + 1], scale=1.0))

    def upd_dve_a(c):
        t, tk = cl[c], ticks[c]
        label[0] = f"{'AB'[c]}.upd"
        wait("dve", "act", tk["sqy"])
        tk["t1"] = emit("dve", nc.vector.tensor_tensor(t["t1"][:], t["sqx"][:], t["sqy"][:], AL.add))
        wait("dve", "act", tk["sqz"])
        tk["d"] = emit("dve", nc.vector.tensor_tensor(t["dd"][:], t["t1"][:], t["sqz"][:], AL.add))

    def upd_dve_b(c):
        t, tk = cl[c], ticks[c]
        label[0] = f"{'AB'[c]}.upd"
        tk["min"] = emit("dve", nc.vector.tensor_tensor(t["dist"][:], t["dist"][:], t["dd"][:], AL.min))
        tk["rowmax"] = emit("dve", nc.vector.reduce_max(t["rowmax"][:, 0:1], t["dist"][:], axis=AX.X))

    def argmax_rmt(c):
        t, tk = cl[c], ticks[c]
        label[0] = f"{'AB'[c]}.arg"
        wait("pe", "dve", tk["rowmax"])
        tk["rmt"] = emit("pe", nc.tensor.matmul(t["rmt_ps"][:], t["rowmax"][:, 0:1], ident[:], start=True, stop=True))

    def argmax_m(c):
        t, tk = cl[c], ticks[c]
        label[0] = f"{'AB'[c]}.arg"
        wait("dve", "pe", tk["rmt"])
        tk["m"] = emit("dve", nc.vector.reduce_max(t["msb"][0:1, 0:1], t["rmt_ps"][0:1, :], axis=AX.X))

    def argmax_mb(c):
        t, tk = cl[c], ticks[c]
        label[0] = f"{'AB'[c]}.arg"
        wait("pe", "dve", tk["m"])
        tk["mb"] = emit("pe", nc.tensor.matmul(t["mb_ps"][:], onesrow[0:1, :], t["msb"][0:1, 0:1], start=True, stop=True))

    def argmax_mask(c):
        t, tk = cl[c], ticks[c]
        label[0] = f"{'AB'[c]}.arg"
        wait("dve", "pe", tk["mb"])
        tk["mask"] = emit("dve", nc.vector.tensor_tensor(
            t["mask"][:], t["dist"][:], t["mb_ps"][:, 0:1].broadcast_to((128, 128)), AL.is_equal))

    def gather_mul(c):
        t, tk = cl[c], ticks[c]
        label[0] = f"{'AB'[c]}.gat"
        mask_rep = t["mask"][:].rearrange("p (a f) -> p a f", a=1).broadcast_to((128, 3, 128))
        tk["mul"] = emit("dve", nc.vector.tensor_tensor(t["scr4"][:], t["mega"][:], mask_rep, AL.mult))

    def gather_red(c):
        t, tk = cl[c], ticks[c]
        label[0] = f"{'AB'[c]}.gat"
        tk["rs"] = emit("dve", nc.vector.tensor_reduce(
            t["rs"][:, 0:3], t["scr4"][:].rearrange("p (k f) -> p k f", k=3), axis=AX.X, op=AL.add))

    def phase_gather(c, s):
        gather_mul(c)
        gather_red(c)

    def tail_pe(c, s):
        t, tk = cl[c], ticks[c]
        label[0] = f"{'AB'[c]}.tai"
        wait("pe", "dve", tk["rs"])
        tk["negbc"] = emit("pe", nc.tensor.matmul(t["negbc_ps"][:], negones[:], t["rs"][:, 0:3], start=True, stop=True))

    def tail_act(c, s):
        t, tk = cl[c], ticks[c]
        label[0] = f"{'AB'[c]}.tai"
        wait("act", "pe", tk["negbc"])
        tk["bias"] = emit("act", nc.scalar.copy(t["biassb"][:], t["negbc_ps"][:, 0:3]))
        tk["out"] = emit("act", nc.scalar.copy(t["outrow"][0:1, 3 * s:3 * s + 3], t["negbc_ps"][0:1, 0:3]))

    def phase_update(c, s):
        upd_head(c, s)
        upd_dve_a(c)
        upd_dve_b(c)

    def phase_argmax(c, s):
        argmax_rmt(c)
        argmax_m(c)
        argmax_mb(c)
        argmax_mask(c)

    def phase_tail(c, s):
        tail_pe(c, s)
        tail_act(c, s)

    if n_clouds == 2:
        # software-pipelined at op granularity: cloud B runs ~half a step
        # behind cloud A; B's update DVE ops are slotted into A's
        # transpose/broadcast PSUM round-trip gaps.
        A, Bc = 0, 1

        def steady(s, first=False):
            upd_head(A, s)
            if not first:
                tail_pe(Bc, s - 1)
            upd_dve_a(A)
            upd_dve_b(A)
            if not first:
                tail_act(Bc, s - 1)
            argmax_rmt(A)
            argmax_m(A)
            upd_head(Bc, s)
            upd_dve_a(Bc)
            argmax_mb(A)
            argmax_mask(A)
            upd_dve_b(Bc)
            gather_mul(A)
            argmax_rmt(Bc)
            argmax_m(Bc)
            gather_red(A)
            argmax_mb(Bc)
            tail_pe(A, s)
            tail_act(A, s)
            argmax_mask(Bc)
            phase_gather(Bc, s)

        steady(1, first=True)
        for s in range(2, S):
            steady(s)
        tail_pe(Bc, S - 1)
        tail_act(Bc, S - 1)
    else:
        for s in range(1, S):
            for c in range(n_clouds):
                phase_update(c, s)
            for c in range(n_clouds):
                phase_argmax(c, s)
            for c in range(n_clouds):
                phase_gather(c, s)
            for c in range(n_clouds):
                phase_tail(c, s)

    for c in range(n_clouds):
        wait("gp", "act", ticks[c]["out"])
        emit("gp", nc.gpsimd.dma_start(out_d[c], cl[c]["outrow"][0:1, :]), 16)

    es.close()
    return nc


def _make_inputs(pos_pair):
    ncl = pos_pair.shape[0]
    mega = np.empty((ncl, 128, 384), np.float32)
    bias0 = np.empty((ncl, 128, 3), np.float32)
    for c in range(ncl):
        for j in range(3):
            mega[c, :, j * 128:(j + 1) * 128] = pos_pair[c, :, j].reshape(128, 128)
        bias0[c] = -pos_pair[c, 0]
    return {
        "mega": mega,
        "bias0": bias0,
        "ident": np.eye(128, dtype=np.float32),
        "onesrow": np.ones((1, 128), np.float32),
        "negones": np.full((128, 128), -1.0, np.float32),
    }


def _get_nc():
    if "nc" not in _CACHE:
        _CACHE["nc"] = _build_fps_kernel()
    return _CACHE["nc"]


def run_on_cores(pos, **spmd_kwargs):
    """pos [16, 16384, 3] f32 -> (idx [16*2048] int32, BassKernelResults)."""
    pos = np.ascontiguousarray(np.asarray(pos, dtype=np.float32))
    assert pos.shape == (B, N, 3)
    nc = _get_nc()
    in_maps = [_make_inputs(pos[N_CLOUDS * c: N_CLOUDS * (c + 1)]) for c in range(N_CORES)]
    res = run_bass_kernel_spmd(nc, in_maps, core_ids=list(range(N_CORES)), **spmd_kwargs)
    idx = np.empty((B, S), np.int32)
    for core in range(N_CORES):
        outrow = res.results[core]["outrow"]  # [n_clouds, 3S]
        for c in range(N_CLOUDS):
            b = N_CLOUDS * core + c
            coords = np.ascontiguousarray(-outrow[c].reshape(S, 3))
            lut = {}
            pb = np.ascontiguousarray(pos[b])
            for n in range(pb.shape[0]):
                lut[pb[n].tobytes()] = n
            loc = np.empty(S, np.int32)
            loc[0] = 0
            for si in range(1, S):
                loc[si] = lut[coords[si].tobytes()]
            idx[b] = loc + b * N
    return idx.reshape(-1), res


def kernel(pos):
    idx, _ = run_on_cores(pos)
    return idx



# revision 13
# speedup vs baseline: 1.1237x; 1.0444x over previous
"""Farthest Point Sampling (FPS) Bass/TRN2 kernel.

Problem: pos [16, 16384, 3] f32 -> indices [16*2048] int32 (exact FPS,
start index 0, ratio 1/8), bit-exact trajectory vs the f32 reference.

Sharding: batch 16 clouds -> 8 NeuronCores, 2 clouds per core (data
parallel, no cross-core communication). Each cloud is laid out as
[128 partitions, 128 free] (point n -> (n//128, n%128)).

Per FPS step per cloud (raw bass, manual semaphores):
  ACT : SQX/SQY/SQZ = Square(coord + bias)       bias = -c (per-partition AP)
  DVE : t1 = SQX+SQY; d = t1+SQZ; DIST = min(DIST, d); rowmax = max_f(DIST)
  PE  : rmT = rowmax^T                           (matmul vs identity -> PSUM)
  DVE : M = max(rmT)                             [1,1]
  PE  : Mb = ones_row^T @ M                      broadcast M -> [128,1] PSUM
  DVE : MASK = is_equal(DIST, Mb)                single-hot (no ties, verified)
  DVE : SCR4 = MEGA4 * MASK(x4);  RS = reduce_add -> [128,4]  (MEGA4 = [X|Y|Z|GIOTA])
  PE  : NEGBC = (-1)^T128 @ RS -> PSUM [128,4]   = (-cx,-cy,-cz,-n*) everywhere
  ACT : BIAS = NEGBC[:,0:3] -> SBUF;  OUTROW[0, 4s:4s+4] = NEGBC[0,:]
Host decodes n* = -OUTROW[4s+3]."""

import numpy as np
from contextlib import ExitStack

import concourse.bass as bass
import concourse.mybir as mybir
from concourse.bass_utils import run_bass_kernel_spmd

AT = mybir.ActivationFunctionType
AL = mybir.AluOpType
AX = mybir.AxisListType
F32 = mybir.dt.float32

B, N, S = 16, 16384, 2048
N_CORES = 8
N_CLOUDS = 2  # per core
BIG = 1.0e10

_CACHE = {}
LABELS = {}


def _build_fps_kernel(S=S, n_clouds=N_CLOUDS):
    nc = bass.Bass(trn_type="TRN2", detect_race_conditions=False)
    mega_d = nc.dram_tensor("mega", [n_clouds, 128, 384], F32, kind="ExternalInput")
    bias0_d = nc.dram_tensor("bias0", [n_clouds, 128, 3], F32, kind="ExternalInput")
    ident_d = nc.dram_tensor("ident", [128, 128], F32, kind="ExternalInput")
    onesrow_d = nc.dram_tensor("onesrow", [1, 128], F32, kind="ExternalInput")
    negones_d = nc.dram_tensor("negones", [128, 128], F32, kind="ExternalInput")
    out_d = nc.dram_tensor("outrow", [n_clouds, 3 * S], F32, kind="ExternalOutput")

    es = ExitStack()
    counter = [0]

    def sb(shape):
        counter[0] += 1
        return es.enter_context(nc.sbuf_tensor(f"sb{counter[0]}", shape, F32))

    def ps(shape):
        counter[0] += 1
        return es.enter_context(nc.psum_tensor(f"ps{counter[0]}", shape, F32))

    ident = sb([128, 128])
    onesrow = sb([1, 128])
    negones = sb([128, 128])

    cl = []
    for c in range(n_clouds):
        cl.append(dict(
            mega=sb([128, 384]),
            dist=sb([128, 128]),
            sqx=sb([128, 128]), sqy=sb([128, 128]), sqz=sb([128, 128]),
            t1=sb([128, 128]), dd=sb([128, 128]),
            mask=sb([128, 128]),
            scr4=sb([128, 128]),
            rs=sb([128, 3]),
            rowmax=sb([128, 1]),
            msb=sb([1, 1]),
            biassb=sb([128, 3]),
            outrow=sb([1, 3 * S]),
            rmt_ps=ps([1, 128]),
            mb_ps=ps([128, 1]),
            negbc_ps=ps([128, 3]),
        ))

    sem_act = es.enter_context(nc.semaphore())
    sem_dve = es.enter_context(nc.semaphore())
    sem_pe = es.enter_context(nc.semaphore())
    sem_gp = es.enter_context(nc.semaphore())

    sems = {"act": sem_act, "dve": sem_dve, "pe": sem_pe, "gp": sem_gp}
    engines = {"act": nc.scalar, "dve": nc.vector, "pe": nc.tensor, "gp": nc.gpsimd}
    count = {k: 0 for k in sems}
    waited = {(a, b): 0 for a in sems for b in sems}
    label = [None]

    def emit(eng, instr, inc=1):
        instr.then_inc(sems[eng], inc)
        count[eng] += inc
        if label[0] is not None:
            try:
                LABELS[instr.ins.name] = label[0]
            except Exception:
                pass
        return count[eng]

    def wait(consumer, producer, tick):
        if tick is None or consumer == producer:
            return
        if waited[(consumer, producer)] < tick:
            engines[consumer].wait_ge(sems[producer], tick)
            waited[(consumer, producer)] = tick

    for c in range(n_clouds):
        emit("gp", nc.gpsimd.dma_start(cl[c]["mega"][:], mega_d[c]), 16)
        emit("gp", nc.gpsimd.dma_start(cl[c]["biassb"][:], bias0_d[c]), 16)
    emit("gp", nc.gpsimd.dma_start(ident[:], ident_d[:]), 16)
    emit("gp", nc.gpsimd.dma_start(onesrow[:], onesrow_d[:]), 16)
    emit("gp", nc.gpsimd.dma_start(negones[:], negones_d[:]), 16)
    dma0 = count["gp"]
    for c in range(n_clouds):
        wait("dve", "gp", dma0)
        emit("dve", nc.vector.memset(cl[c]["dist"][:], BIG))
        emit("dve", nc.vector.memset(cl[c]["outrow"][:], 0.0))
    wait("act", "gp", dma0)
    wait("pe", "gp", dma0)

    ticks = [dict() for _ in range(n_clouds)]

    def upd_head(c, s, jlist=(0, 1, 2)):
        t, tk = cl[c], ticks[c]
        label[0] = f"{'AB'[c]}.upd"
        names = ("sqx", "sqy", "sqz")
        for j in jlist:
            tk[names[j]] = emit("act", nc.scalar.activation(
                t[names[j]][:], t["mega"][:, j * 128:(j + 1) * 128], AT.Square,
                bias=t["biassb"][:, j:j + 1], scale=1.0))

    def upd_dve_a(c):
        t, tk = cl[c], ticks[c]
        label[0] = f"{'AB'[c]}.upd"
        wait("dve", "act", tk["sqy"])
        tk["t1"] = emit("dve", nc.vector.tensor_tensor(t["t1"][:], t["sqx"][:], t["sqy"][:], AL.add))
        wait("dve", "act", tk["sqz"])
        tk["d"] = emit("dve", nc.vector.tensor_tensor(t["dd"][:], t["t1"][:], t["sqz"][:], AL.add))

    def upd_dve_b(c):
        t, tk = cl[c], ticks[c]
        label[0] = f"{'AB'[c]}.upd"
        tk["min"] = emit("dve", nc.vector.tensor_tensor(t["dist"][:], t["dist"][:], t["dd"][:], AL.min))
        tk["rowmax"] = emit("dve", nc.vector.reduce_max(t["rowmax"][:, 0:1], t["dist"][:], axis=AX.X))

    def argmax_rmt(c):
        t, tk = cl[c], ticks[c]
        label[0] = f"{'AB'[c]}.arg"
        wait("pe", "dve", tk["rowmax"])
        tk["rmt"] = emit("pe", nc.tensor.matmul(t["rmt_ps"][:], t["rowmax"][:, 0:1], ident[:], start=True, stop=True))

    def argmax_m(c):
        t, tk = cl[c], ticks[c]
        label[0] = f"{'AB'[c]}.arg"
        wait("dve", "pe", tk["rmt"])
        tk["m"] = emit("dve", nc.vector.reduce_max(t["msb"][0:1, 0:1], t["rmt_ps"][0:1, :], axis=AX.X))

    def argmax_mb(c):
        t, tk = cl[c], ticks[c]
        label[0] = f"{'AB'[c]}.arg"
        wait("pe", "dve", tk["m"])
        tk["mb"] = emit("pe", nc.tensor.matmul(t["mb_ps"][:], onesrow[0:1, :], t["msb"][0:1, 0:1], start=True, stop=True))

    def argmax_mask(c):
        t, tk = cl[c], ticks[c]
        label[0] = f"{'AB'[c]}.arg"
        wait("dve", "pe", tk["mb"])
        tk["mask"] = emit("dve", nc.vector.tensor_tensor(
            t["mask"][:], t["dist"][:], t["mb_ps"][:, 0:1].broadcast_to((128, 128)), AL.is_equal))

    def gather_stt(c, jlist=(0, 1, 2)):
        t, tk = cl[c], ticks[c]
        label[0] = f"{'AB'[c]}.gat"
        for j in jlist:
            tk["rs"] = emit("dve", nc.vector.scalar_tensor_tensor(
                t["scr4"][:], t["mega"][:, j * 128:(j + 1) * 128], 1.0, t["mask"][:],
                AL.mult, AL.mult, accum_out=t["rs"][:, j:j + 1]))

    def tail_pe(c, s):
        t, tk = cl[c], ticks[c]
        label[0] = f"{'AB'[c]}.tai"
        wait("pe", "dve", tk["rs"])
        tk["negbc"] = emit("pe", nc.tensor.matmul(t["negbc_ps"][:], negones[:], t["rs"][:, 0:3], start=True, stop=True))

    def tail_act(c, s):
        t, tk = cl[c], ticks[c]
        label[0] = f"{'AB'[c]}.tai"
        wait("act", "pe", tk["negbc"])
        tk["bias"] = emit("act", nc.scalar.copy(t["biassb"][:], t["negbc_ps"][:, 0:3]))
        tk["out"] = emit("act", nc.scalar.copy(t["outrow"][0:1, 3 * s:3 * s + 3], t["negbc_ps"][0:1, 0:3]))

    def phase_update(c, s):
        upd_head(c, s)
        upd_dve_a(c)
        upd_dve_b(c)

    def phase_argmax(c, s):
        argmax_rmt(c)
        argmax_m(c)
        argmax_mb(c)
        argmax_mask(c)

    def phase_tail(c, s):
        tail_pe(c, s)
        tail_act(c, s)

    if n_clouds == 2:
        # B runs ~a quarter-step behind A; B's gather for step s-1 fills
        # A's transpose/broadcast PSUM round-trip gaps in iteration s.
        A, Bc = 0, 1

        def steady(s, first=False):
            upd_head(A, s)            # ACT A-sq(s)
            upd_dve_a(A)              # DVE A-t1, A-d
            upd_dve_b(A)              # DVE A-min, A-rowmax
            argmax_rmt(A)             # PE
            if not first:
                gather_stt(Bc, (0, 1))   # DVE fills A-rmT wait
            argmax_m(A)               # DVE A-m
            argmax_mb(A)              # PE
            if not first:
                gather_stt(Bc, (2,))     # DVE fills A-Mb wait
                tail_pe(Bc, s - 1)       # PE B-negbc
            argmax_mask(A)            # DVE A-mask
            if not first:
                tail_act(Bc, s - 1)      # ACT B-bias, B-out (after A-sq)
            gather_stt(A)             # DVE A-stt x3
            tail_pe(A, s)             # PE A-negbc
            upd_head(Bc, s, (0, 1))   # ACT B-sqx, B-sqy
            tail_act(A, s)            # ACT A-bias, A-out
            upd_head(Bc, s, (2,))     # ACT B-sqz
            upd_dve_a(Bc)             # DVE B-t1, B-d
            upd_dve_b(Bc)             # DVE B-min, B-rowmax
            argmax_rmt(Bc)            # PE
            argmax_m(Bc)              # DVE B-m
            argmax_mb(Bc)             # PE
            argmax_mask(Bc)           # DVE B-mask (gather deferred to next iter)

        steady(1, first=True)
        for s in range(2, S):
            steady(s)
        gather_stt(Bc)
        tail_pe(Bc, S - 1)
        tail_act(Bc, S - 1)
    else:
        raise NotImplementedError

    for c in range(n_clouds):
        wait("gp", "act", ticks[c]["out"])
        emit("gp", nc.gpsimd.dma_start(out_d[c], cl[c]["outrow"][0:1, :]), 16)

    es.close()
    return nc


def _make_inputs(pos_pair):
    ncl = pos_pair.shape[0]
    mega = np.empty((ncl, 128, 384), np.float32)
    bias0 = np.empty((ncl, 128, 3), np.float32)
    for c in range(ncl):
        for j in range(3):
            mega[c, :, j * 128:(j + 1) * 128] = pos_pair[c, :, j].reshape(128, 128)
        bias0[c] = -pos_pair[c, 0]
    return {
        "mega": mega,
        "bias0": bias0,
        "ident": np.eye(128, dtype=np.float32),
        "onesrow": np.ones((1, 128), np.float32),
        "negones": np.full((128, 128), -1.0, np.float32),
    }


def _get_nc():
    if "nc" not in _CACHE:
        _CACHE["nc"] = _build_fps_kernel()
    return _CACHE["nc"]


def run_on_cores(pos, **spmd_kwargs):
    """pos [16, 16384, 3] f32 -> (idx [16*2048] int32, BassKernelResults)."""
    pos = np.ascontiguousarray(np.asarray(pos, dtype=np.float32))
    assert pos.shape == (B, N, 3)
    nc = _get_nc()
    in_maps = [_make_inputs(pos[N_CLOUDS * c: N_CLOUDS * (c + 1)]) for c in range(N_CORES)]
    res = run_bass_kernel_spmd(nc, in_maps, core_ids=list(range(N_CORES)), **spmd_kwargs)
    idx = np.empty((B, S), np.int32)
    for core in range(N_CORES):
        outrow = res.results[core]["outrow"]  # [n_clouds, 3S]
        for c in range(N_CLOUDS):
            b = N_CLOUDS * core + c
            coords = np.ascontiguousarray(-outrow[c].reshape(S, 3))
            pb = np.ascontiguousarray(pos[b])
            lut = {pb[n].tobytes(): n for n in range(pb.shape[0])}
            loc = np.empty(S, np.int32)
            loc[0] = 0
            for si in range(1, S):
                loc[si] = lut.get(coords[si].tobytes(), -1)
            idx[b] = loc + b * N
    return idx.reshape(-1), res


def kernel(pos):
    idx, _ = run_on_cores(pos)
    return idx



# revision 14
# speedup vs baseline: 1.2979x; 1.1551x over previous
"""Farthest Point Sampling (FPS) Bass/TRN2 kernel.

Problem: pos [16, 16384, 3] f32 -> indices [16*2048] int32 (exact FPS,
start index 0, ratio 1/8), bit-exact trajectory vs the f32 reference.

Sharding: batch 16 clouds -> 8 NeuronCores, 2 clouds per core (data
parallel, no cross-core communication). Each cloud is laid out as
[128 partitions, 128 free] (point n -> (n//128, n%128)).

Per FPS step per cloud (raw bass, manual semaphores):
  ACT : SQX/SQY/SQZ = Square(coord + bias)       bias = -c (per-partition AP)
  DVE : t1 = SQX+SQY; d = t1+SQZ; DIST = min(DIST, d); rowmax = max_f(DIST)
  PE  : rmT = rowmax^T                           (matmul vs identity -> PSUM)
  DVE : M = max(rmT)                             [1,1]
  PE  : Mb = ones_row^T @ M                      broadcast M -> [128,1] PSUM
  DVE : MASK = is_equal(DIST, Mb)                single-hot (no ties, verified)
  DVE : SCR4 = MEGA4 * MASK(x4);  RS = reduce_add -> [128,4]  (MEGA4 = [X|Y|Z|GIOTA])
  PE  : NEGBC = (-1)^T128 @ RS -> PSUM [128,4]   = (-cx,-cy,-cz,-n*) everywhere
  ACT : BIAS = NEGBC[:,0:3] -> SBUF;  OUTROW[0, 4s:4s+4] = NEGBC[0,:]
Host decodes n* = -OUTROW[4s+3]."""

import numpy as np
from contextlib import ExitStack

import concourse.bass as bass
import concourse.mybir as mybir
from concourse.bass_utils import run_bass_kernel_spmd

AT = mybir.ActivationFunctionType
AL = mybir.AluOpType
AX = mybir.AxisListType
F32 = mybir.dt.float32

B, N, S = 16, 16384, 2048
N_CORES = 8
N_CLOUDS = 2  # per core
BIG = 1.0e10

_CACHE = {}
LABELS = {}


def _build_fps_kernel(S=S, n_clouds=N_CLOUDS):
    nc = bass.Bass(trn_type="TRN2", detect_race_conditions=False)
    mega_d = nc.dram_tensor("mega", [n_clouds, 128, 384], F32, kind="ExternalInput")
    bias0_d = nc.dram_tensor("bias0", [n_clouds, 128, 3], F32, kind="ExternalInput")
    ident_d = nc.dram_tensor("ident", [128, 128], F32, kind="ExternalInput")
    onesrow_d = nc.dram_tensor("onesrow", [1, 128], F32, kind="ExternalInput")
    negones_d = nc.dram_tensor("negones", [128, 128], F32, kind="ExternalInput")
    out_d = nc.dram_tensor("outrow", [n_clouds, 3 * S], F32, kind="ExternalOutput")

    es = ExitStack()
    counter = [0]

    def sb(shape):
        counter[0] += 1
        return es.enter_context(nc.sbuf_tensor(f"sb{counter[0]}", shape, F32))

    def ps(shape):
        counter[0] += 1
        return es.enter_context(nc.psum_tensor(f"ps{counter[0]}", shape, F32))

    ident = sb([128, 128])
    onesrow = sb([1, 128])
    negones = sb([128, 128])

    cl = []
    for c in range(n_clouds):
        cl.append(dict(
            mega=sb([128, 384]),
            dist=sb([128, 128]),
            sqx=sb([128, 128]), sqy=sb([128, 128]), sqz=sb([128, 128]),
            t1=sb([128, 128]), dd=sb([128, 128]),
            mask=sb([128, 128]),
            scr4=sb([128, 128]),
            rs=sb([128, 3]),
            rowmax=sb([128, 1]),
            msb=sb([1, 1]),
            biassb=sb([128, 3]),
            outrow=sb([1, 3 * S]),
            rmt_ps=ps([1, 128]),
            mb_ps=ps([128, 1]),
            negbc_ps=ps([128, 3]),
        ))

    sem_act = es.enter_context(nc.semaphore())
    sem_dve = es.enter_context(nc.semaphore())
    sem_pe = es.enter_context(nc.semaphore())
    sem_gp = es.enter_context(nc.semaphore())

    sems = {"act": sem_act, "dve": sem_dve, "pe": sem_pe, "gp": sem_gp}
    engines = {"act": nc.scalar, "dve": nc.vector, "pe": nc.tensor, "gp": nc.gpsimd}
    count = {k: 0 for k in sems}
    waited = {(a, b): 0 for a in sems for b in sems}
    label = [None]

    def emit(eng, instr, inc=1):
        instr.then_inc(sems[eng], inc)
        count[eng] += inc
        if label[0] is not None:
            try:
                LABELS[instr.ins.name] = label[0]
            except Exception:
                pass
        return count[eng]

    def wait(consumer, producer, tick):
        if tick is None or consumer == producer:
            return
        if waited[(consumer, producer)] < tick:
            engines[consumer].wait_ge(sems[producer], tick)
            waited[(consumer, producer)] = tick

    for c in range(n_clouds):
        emit("gp", nc.gpsimd.dma_start(cl[c]["mega"][:], mega_d[c]), 16)
        emit("gp", nc.gpsimd.dma_start(cl[c]["biassb"][:], bias0_d[c]), 16)
    emit("gp", nc.gpsimd.dma_start(ident[:], ident_d[:]), 16)
    emit("gp", nc.gpsimd.dma_start(onesrow[:], onesrow_d[:]), 16)
    emit("gp", nc.gpsimd.dma_start(negones[:], negones_d[:]), 16)
    dma0 = count["gp"]
    for c in range(n_clouds):
        wait("dve", "gp", dma0)
        emit("dve", nc.vector.memset(cl[c]["dist"][:], BIG))
        emit("dve", nc.vector.memset(cl[c]["outrow"][:], 0.0))
    wait("act", "gp", dma0)
    wait("pe", "gp", dma0)

    ticks = [dict() for _ in range(n_clouds)]

    def upd_head(c, s, jlist=(0, 1, 2)):
        t, tk = cl[c], ticks[c]
        label[0] = f"{'AB'[c]}.upd"
        names = ("sqx", "sqy", "sqz")
        for j in jlist:
            tk[names[j]] = emit("act", nc.scalar.activation(
                t[names[j]][:], t["mega"][:, j * 128:(j + 1) * 128], AT.Square,
                bias=t["biassb"][:, j:j + 1], scale=1.0))

    def upd_dve_a(c):
        t, tk = cl[c], ticks[c]
        label[0] = f"{'AB'[c]}.upd"
        wait("dve", "act", tk["sqy"])
        tk["t1"] = emit("dve", nc.vector.tensor_tensor(t["t1"][:], t["sqx"][:], t["sqy"][:], AL.add))
        wait("dve", "act", tk["sqz"])
        tk["d"] = emit("dve", nc.vector.tensor_tensor(t["dd"][:], t["t1"][:], t["sqz"][:], AL.add))

    def upd_dve_b(c):
        t, tk = cl[c], ticks[c]
        label[0] = f"{'AB'[c]}.upd"
        tk["min"] = emit("dve", nc.vector.tensor_tensor(t["dist"][:], t["dist"][:], t["dd"][:], AL.min))
        tk["rowmax"] = emit("dve", nc.vector.reduce_max(t["rowmax"][:, 0:1], t["dist"][:], axis=AX.X))

    def argmax_rmt(c):
        t, tk = cl[c], ticks[c]
        label[0] = f"{'AB'[c]}.arg"
        wait("pe", "dve", tk["rowmax"])
        tk["rmt"] = emit("pe", nc.tensor.matmul(t["rmt_ps"][:], t["rowmax"][:, 0:1], ident[:], start=True, stop=True))

    def argmax_m(c):
        t, tk = cl[c], ticks[c]
        label[0] = f"{'AB'[c]}.arg"
        wait("dve", "pe", tk["rmt"])
        tk["m"] = emit("dve", nc.vector.reduce_max(t["msb"][0:1, 0:1], t["rmt_ps"][0:1, :], axis=AX.X))

    def argmax_mb(c):
        t, tk = cl[c], ticks[c]
        label[0] = f"{'AB'[c]}.arg"
        wait("pe", "dve", tk["m"])
        tk["mb"] = emit("pe", nc.tensor.matmul(t["mb_ps"][:], onesrow[0:1, :], t["msb"][0:1, 0:1], start=True, stop=True))

    def argmax_mask(c):
        t, tk = cl[c], ticks[c]
        label[0] = f"{'AB'[c]}.arg"
        wait("dve", "pe", tk["mb"])
        tk["mask"] = emit("dve", nc.vector.tensor_tensor(
            t["mask"][:], t["dist"][:], t["mb_ps"][:, 0:1].broadcast_to((128, 128)), AL.is_equal))

    def gather_stt(c, jlist=(0, 1, 2)):
        t, tk = cl[c], ticks[c]
        label[0] = f"{'AB'[c]}.gat"
        for j in jlist:
            tk["rs"] = emit("dve", nc.vector.scalar_tensor_tensor(
                t["scr4"][:], t["mega"][:, j * 128:(j + 1) * 128], 1.0, t["mask"][:],
                AL.mult, AL.mult, accum_out=t["rs"][:, j:j + 1]))

    def tail_pe(c, s):
        t, tk = cl[c], ticks[c]
        label[0] = f"{'AB'[c]}.tai"
        wait("pe", "dve", tk["rs"])
        tk["negbc"] = emit("pe", nc.tensor.matmul(t["negbc_ps"][:], negones[:], t["rs"][:, 0:3], start=True, stop=True))

    def tail_act(c, s):
        t, tk = cl[c], ticks[c]
        label[0] = f"{'AB'[c]}.tai"
        wait("act", "pe", tk["negbc"])
        tk["bias"] = emit("act", nc.scalar.copy(t["biassb"][:], t["negbc_ps"][:, 0:3]))
        tk["out"] = emit("act", nc.scalar.copy(t["outrow"][0:1, 3 * s:3 * s + 3], t["negbc_ps"][0:1, 0:3]))

    def phase_update(c, s):
        upd_head(c, s)
        upd_dve_a(c)
        upd_dve_b(c)

    def phase_argmax(c, s):
        argmax_rmt(c)
        argmax_m(c)
        argmax_mb(c)
        argmax_mask(c)

    def phase_tail(c, s):
        tail_pe(c, s)
        tail_act(c, s)

    if n_clouds == 2:
        # Rotated pipeline: entering steady(s), A's update(s) is complete
        # and B's mask(s-1) is complete. B's gather(s-1) fills A's argmax
        # PSUM round trips; A's update(s+1) fills B's argmax round trips.
        A, Bc = 0, 1

        def steady(s, first=False):
            argmax_rmt(A)             # PE A-rmT(s)
            if not first:
                gather_stt(Bc)           # DVE B-stt x3 (s-1): fills A-rmT wait
            argmax_m(A)               # DVE A-m
            argmax_mb(A)              # PE A-Mb
            if not first:
                tail_pe(Bc, s - 1)       # PE B-negbc
            argmax_mask(A)            # DVE A-mask
            if not first:
                tail_act(Bc, s - 1)      # ACT B-bias, B-out
            gather_stt(A)             # DVE A-stt x3 (s)
            tail_pe(A, s)             # PE A-negbc
            upd_head(Bc, s, (0, 1))   # ACT B-sqx, B-sqy (after B-bias)
            tail_act(A, s)            # ACT A-bias, A-out
            upd_head(Bc, s, (2,))     # ACT B-sqz
            upd_head(A, s + 1)        # ACT A-sq x3 (s+1), after A-bias
            upd_dve_a(Bc)             # DVE B-t1, B-d
            upd_dve_b(Bc)             # DVE B-min, B-rowmax
            argmax_rmt(Bc)            # PE B-rmT
            upd_dve_a(A)              # DVE A-t1, A-d (s+1): fills B-rmT wait
            argmax_m(Bc)              # DVE B-m
            argmax_mb(Bc)             # PE B-Mb
            upd_dve_b(A)              # DVE A-min, A-rowmax (s+1): fills B-Mb wait
            argmax_mask(Bc)           # DVE B-mask

        # prologue: A's update for step 1
        upd_head(A, 1)
        upd_dve_a(A)
        upd_dve_b(A)
        steady(1, first=True)
        for s in range(2, S):
            steady(s)
        # epilogue: B's step S-1 gather + tail
        gather_stt(Bc)
        tail_pe(Bc, S - 1)
        tail_act(Bc, S - 1)
    else:
        raise NotImplementedError

    for c in range(n_clouds):
        wait("gp", "act", ticks[c]["out"])
        emit("gp", nc.gpsimd.dma_start(out_d[c], cl[c]["outrow"][0:1, :]), 16)

    es.close()
    return nc


def _make_inputs(pos_pair):
    ncl = pos_pair.shape[0]
    mega = np.empty((ncl, 128, 384), np.float32)
    bias0 = np.empty((ncl, 128, 3), np.float32)
    for c in range(ncl):
        for j in range(3):
            mega[c, :, j * 128:(j + 1) * 128] = pos_pair[c, :, j].reshape(128, 128)
        bias0[c] = -pos_pair[c, 0]
    return {
        "mega": mega,
        "bias0": bias0,
        "ident": np.eye(128, dtype=np.float32),
        "onesrow": np.ones((1, 128), np.float32),
        "negones": np.full((128, 128), -1.0, np.float32),
    }


def _get_nc():
    if "nc" not in _CACHE:
        _CACHE["nc"] = _build_fps_kernel()
    return _CACHE["nc"]


def run_on_cores(pos, **spmd_kwargs):
    """pos [16, 16384, 3] f32 -> (idx [16*2048] int32, BassKernelResults)."""
    pos = np.ascontiguousarray(np.asarray(pos, dtype=np.float32))
    assert pos.shape == (B, N, 3)
    nc = _get_nc()
    in_maps = [_make_inputs(pos[N_CLOUDS * c: N_CLOUDS * (c + 1)]) for c in range(N_CORES)]
    res = run_bass_kernel_spmd(nc, in_maps, core_ids=list(range(N_CORES)), **spmd_kwargs)
    idx = np.empty((B, S), np.int32)
    for core in range(N_CORES):
        outrow = res.results[core]["outrow"]  # [n_clouds, 3S]
        for c in range(N_CLOUDS):
            b = N_CLOUDS * core + c
            coords = np.ascontiguousarray(-outrow[c].reshape(S, 3))
            pb = np.ascontiguousarray(pos[b])
            lut = {pb[n].tobytes(): n for n in range(pb.shape[0])}
            loc = np.empty(S, np.int32)
            loc[0] = 0
            for si in range(1, S):
                loc[si] = lut.get(coords[si].tobytes(), -1)
            idx[b] = loc + b * N
    return idx.reshape(-1), res


def kernel(pos):
    idx, _ = run_on_cores(pos)
    return idx



# revision 15
# speedup vs baseline: 1.3021x; 1.0032x over previous
"""Farthest Point Sampling (FPS) Bass/TRN2 kernel.

Problem: pos [16, 16384, 3] f32 -> indices [16*2048] int32 (exact FPS,
start index 0, ratio 1/8), bit-exact trajectory vs the f32 reference.

Sharding: batch 16 clouds -> 8 NeuronCores, 2 clouds per core (data
parallel, no cross-core communication). Each cloud is laid out as
[128 partitions, 128 free] (point n -> (n//128, n%128)).

Per FPS step per cloud (raw bass, manual semaphores):
  ACT : SQX/SQY/SQZ = Square(coord + bias)       bias = -c (per-partition AP)
  DVE : t1 = SQX+SQY; d = t1+SQZ; DIST = min(DIST, d); rowmax = max_f(DIST)
  PE  : rmT = rowmax^T                           (matmul vs identity -> PSUM)
  DVE : M = max(rmT)                             [1,1]
  PE  : Mb = ones_row^T @ M                      broadcast M -> [128,1] PSUM
  DVE : MASK = is_equal(DIST, Mb)                single-hot (no ties, verified)
  DVE : SCR4 = MEGA4 * MASK(x4);  RS = reduce_add -> [128,4]  (MEGA4 = [X|Y|Z|GIOTA])
  PE  : NEGBC = (-1)^T128 @ RS -> PSUM [128,4]   = (-cx,-cy,-cz,-n*) everywhere
  ACT : BIAS = NEGBC[:,0:3] -> SBUF;  OUTROW[0, 4s:4s+4] = NEGBC[0,:]
Host decodes n* = -OUTROW[4s+3]."""

import numpy as np
from contextlib import ExitStack

import concourse.bass as bass
import concourse.mybir as mybir
from concourse.bass_utils import run_bass_kernel_spmd

AT = mybir.ActivationFunctionType
AL = mybir.AluOpType
AX = mybir.AxisListType
F32 = mybir.dt.float32

B, N, S = 16, 16384, 2048
N_CORES = 8
N_CLOUDS = 2  # per core
BIG = 1.0e10

_CACHE = {}
LABELS = {}


def _build_fps_kernel(S=S, n_clouds=N_CLOUDS):
    nc = bass.Bass(trn_type="TRN2", detect_race_conditions=False)
    mega_d = nc.dram_tensor("mega", [n_clouds, 128, 384], F32, kind="ExternalInput")
    bias0_d = nc.dram_tensor("bias0", [n_clouds, 128, 3], F32, kind="ExternalInput")
    ident_d = nc.dram_tensor("ident", [128, 128], F32, kind="ExternalInput")
    onesrow_d = nc.dram_tensor("onesrow", [1, 128], F32, kind="ExternalInput")
    negones_d = nc.dram_tensor("negones", [128, 128], F32, kind="ExternalInput")
    out_d = nc.dram_tensor("outrow", [n_clouds, 3 * S], F32, kind="ExternalOutput")

    es = ExitStack()
    counter = [0]

    def sb(shape):
        counter[0] += 1
        return es.enter_context(nc.sbuf_tensor(f"sb{counter[0]}", shape, F32))

    def ps(shape):
        counter[0] += 1
        return es.enter_context(nc.psum_tensor(f"ps{counter[0]}", shape, F32))

    ident = sb([128, 128])
    onesrow = sb([1, 128])
    negones = sb([128, 128])

    cl = []
    for c in range(n_clouds):
        cl.append(dict(
            mega=sb([128, 384]),
            dist=sb([128, 128]),
            sqx=sb([128, 128]), sqy=sb([128, 128]), sqz=sb([128, 128]),
            t1=sb([128, 128]), dd=sb([128, 128]),
            mask=sb([128, 128]),
            scr4=sb([128, 128]),
            rs=sb([128, 3]),
            rowmax=sb([128, 1]),
            msb=sb([1, 1]),
            biassb=sb([128, 3]),
            outrow=sb([1, 3 * S]),
            rmt_ps=ps([1, 128]),
            mb_ps=ps([128, 1]),
            negbc_ps=ps([128, 3]),
        ))

    sem_act = es.enter_context(nc.semaphore())
    sem_dve = es.enter_context(nc.semaphore())
    sem_pe = es.enter_context(nc.semaphore())
    sem_gp = es.enter_context(nc.semaphore())

    sems = {"act": sem_act, "dve": sem_dve, "pe": sem_pe, "gp": sem_gp}
    engines = {"act": nc.scalar, "dve": nc.vector, "pe": nc.tensor, "gp": nc.gpsimd}
    count = {k: 0 for k in sems}
    waited = {(a, b): 0 for a in sems for b in sems}
    label = [None]

    def emit(eng, instr, inc=1):
        instr.then_inc(sems[eng], inc)
        count[eng] += inc
        if label[0] is not None:
            try:
                LABELS[instr.ins.name] = label[0]
            except Exception:
                pass
        return count[eng]

    def wait(consumer, producer, tick):
        if tick is None or consumer == producer:
            return
        if waited[(consumer, producer)] < tick:
            engines[consumer].wait_ge(sems[producer], tick)
            waited[(consumer, producer)] = tick

    for c in range(n_clouds):
        emit("gp", nc.gpsimd.dma_start(cl[c]["mega"][:], mega_d[c]), 16)
        emit("gp", nc.gpsimd.dma_start(cl[c]["biassb"][:], bias0_d[c]), 16)
    emit("gp", nc.gpsimd.dma_start(ident[:], ident_d[:]), 16)
    emit("gp", nc.gpsimd.dma_start(onesrow[:], onesrow_d[:]), 16)
    emit("gp", nc.gpsimd.dma_start(negones[:], negones_d[:]), 16)
    dma0 = count["gp"]
    for c in range(n_clouds):
        wait("dve", "gp", dma0)
        emit("dve", nc.vector.memset(cl[c]["dist"][:], BIG))
        emit("dve", nc.vector.memset(cl[c]["outrow"][:], 0.0))
    wait("act", "gp", dma0)
    wait("pe", "gp", dma0)

    ticks = [dict() for _ in range(n_clouds)]

    def upd_head(c, s, jlist=(0, 1, 2)):
        t, tk = cl[c], ticks[c]
        label[0] = f"{'AB'[c]}.upd"
        names = ("sqx", "sqy", "sqz")
        for j in jlist:
            tk[names[j]] = emit("act", nc.scalar.activation(
                t[names[j]][:], t["mega"][:, j * 128:(j + 1) * 128], AT.Square,
                bias=t["biassb"][:, j:j + 1], scale=1.0))

    def upd_dve_a(c):
        t, tk = cl[c], ticks[c]
        label[0] = f"{'AB'[c]}.upd"
        emit("dve", nc.vector.tensor_scalar(
            t["sqy"][:], t["mega"][:, 128:256], t["biassb"][:, 1:2], None, AL.add))
        emit("dve", nc.vector.tensor_tensor(t["sqy"][:], t["sqy"][:], t["sqy"][:], AL.mult))
        wait("dve", "act", tk["sqx"])
        tk["t1"] = emit("dve", nc.vector.tensor_tensor(t["t1"][:], t["sqx"][:], t["sqy"][:], AL.add))
        wait("dve", "act", tk["sqz"])
        tk["d"] = emit("dve", nc.vector.tensor_tensor(t["dd"][:], t["t1"][:], t["sqz"][:], AL.add))

    def upd_dve_b(c):
        t, tk = cl[c], ticks[c]
        label[0] = f"{'AB'[c]}.upd"
        tk["min"] = emit("dve", nc.vector.tensor_tensor(t["dist"][:], t["dist"][:], t["dd"][:], AL.min))
        tk["rowmax"] = emit("dve", nc.vector.reduce_max(t["rowmax"][:, 0:1], t["dist"][:], axis=AX.X))

    def argmax_rmt(c):
        t, tk = cl[c], ticks[c]
        label[0] = f"{'AB'[c]}.arg"
        wait("pe", "dve", tk["rowmax"])
        tk["rmt"] = emit("pe", nc.tensor.matmul(t["rmt_ps"][:], t["rowmax"][:, 0:1], ident[:], start=True, stop=True))

    def argmax_m(c):
        t, tk = cl[c], ticks[c]
        label[0] = f"{'AB'[c]}.arg"
        wait("dve", "pe", tk["rmt"])
        tk["m"] = emit("dve", nc.vector.reduce_max(t["msb"][0:1, 0:1], t["rmt_ps"][0:1, :], axis=AX.X))

    def argmax_mb(c):
        t, tk = cl[c], ticks[c]
        label[0] = f"{'AB'[c]}.arg"
        wait("pe", "dve", tk["m"])
        tk["mb"] = emit("pe", nc.tensor.matmul(t["mb_ps"][:], onesrow[0:1, :], t["msb"][0:1, 0:1], start=True, stop=True))

    def argmax_mask(c):
        t, tk = cl[c], ticks[c]
        label[0] = f"{'AB'[c]}.arg"
        wait("dve", "pe", tk["mb"])
        tk["mask"] = emit("dve", nc.vector.tensor_tensor(
            t["mask"][:], t["dist"][:], t["mb_ps"][:, 0:1].broadcast_to((128, 128)), AL.is_equal))

    def gather_stt(c, jlist=(0, 1, 2)):
        t, tk = cl[c], ticks[c]
        label[0] = f"{'AB'[c]}.gat"
        for j in jlist:
            tk["rs"] = emit("dve", nc.vector.scalar_tensor_tensor(
                t["scr4"][:], t["mega"][:, j * 128:(j + 1) * 128], 1.0, t["mask"][:],
                AL.mult, AL.mult, accum_out=t["rs"][:, j:j + 1]))

    def tail_pe(c, s):
        t, tk = cl[c], ticks[c]
        label[0] = f"{'AB'[c]}.tai"
        wait("pe", "dve", tk["rs"])
        tk["negbc"] = emit("pe", nc.tensor.matmul(t["negbc_ps"][:], negones[:], t["rs"][:, 0:3], start=True, stop=True))

    def tail_act(c, s):
        t, tk = cl[c], ticks[c]
        label[0] = f"{'AB'[c]}.tai"
        wait("act", "pe", tk["negbc"])
        tk["bias"] = emit("act", nc.scalar.copy(t["biassb"][:], t["negbc_ps"][:, 0:3]))
        tk["out"] = emit("act", nc.scalar.copy(t["outrow"][0:1, 3 * s:3 * s + 3], t["negbc_ps"][0:1, 0:3]))

    def phase_update(c, s):
        upd_head(c, s)
        upd_dve_a(c)
        upd_dve_b(c)

    def phase_argmax(c, s):
        argmax_rmt(c)
        argmax_m(c)
        argmax_mb(c)
        argmax_mask(c)

    def phase_tail(c, s):
        tail_pe(c, s)
        tail_act(c, s)

    if n_clouds == 2:
        # Rotated pipeline: entering steady(s), A's update(s) is complete
        # and B's mask(s-1) is complete. B's gather(s-1) fills A's argmax
        # PSUM round trips; A's update(s+1) fills B's argmax round trips.
        A, Bc = 0, 1

        def steady(s, first=False):
            argmax_rmt(A)             # PE A-rmT(s)
            if not first:
                gather_stt(Bc)           # DVE B-stt x3 (s-1): fills A-rmT wait
            argmax_m(A)               # DVE A-m
            argmax_mb(A)              # PE A-Mb
            if not first:
                tail_pe(Bc, s - 1)       # PE B-negbc
            argmax_mask(A)            # DVE A-mask
            if not first:
                tail_act(Bc, s - 1)      # ACT B-bias, B-out
            gather_stt(A)             # DVE A-stt x3 (s)
            tail_pe(A, s)             # PE A-negbc
            upd_head(Bc, s, (0,))     # ACT B-sqx (after B-bias)
            tail_act(A, s)            # ACT A-bias, A-out
            upd_head(Bc, s, (2,))     # ACT B-sqz
            upd_head(A, s + 1, (0, 2))  # ACT A-sqx, A-sqz (s+1), after A-bias
            upd_dve_a(Bc)             # DVE B-t1, B-d
            upd_dve_b(Bc)             # DVE B-min, B-rowmax
            argmax_rmt(Bc)            # PE B-rmT
            upd_dve_a(A)              # DVE A-t1, A-d (s+1): fills B-rmT wait
            argmax_m(Bc)              # DVE B-m
            argmax_mb(Bc)             # PE B-Mb
            upd_dve_b(A)              # DVE A-min, A-rowmax (s+1): fills B-Mb wait
            argmax_mask(Bc)           # DVE B-mask

        # prologue: A's update for step 1
        upd_head(A, 1, (0, 2))
        upd_dve_a(A)
        upd_dve_b(A)
        steady(1, first=True)
        for s in range(2, S):
            steady(s)
        # epilogue: B's step S-1 gather + tail
        gather_stt(Bc)
        tail_pe(Bc, S - 1)
        tail_act(Bc, S - 1)
    else:
        raise NotImplementedError

    for c in range(n_clouds):
        wait("gp", "act", ticks[c]["out"])
        emit("gp", nc.gpsimd.dma_start(out_d[c], cl[c]["outrow"][0:1, :]), 16)

    es.close()
    return nc


def _make_inputs(pos_pair):
    ncl = pos_pair.shape[0]
    mega = np.empty((ncl, 128, 384), np.float32)
    bias0 = np.empty((ncl, 128, 3), np.float32)
    for c in range(ncl):
        for j in range(3):
            mega[c, :, j * 128:(j + 1) * 128] = pos_pair[c, :, j].reshape(128, 128)
        bias0[c] = -pos_pair[c, 0]
    return {
        "mega": mega,
        "bias0": bias0,
        "ident": np.eye(128, dtype=np.float32),
        "onesrow": np.ones((1, 128), np.float32),
        "negones": np.full((128, 128), -1.0, np.float32),
    }


def _get_nc():
    if "nc" not in _CACHE:
        _CACHE["nc"] = _build_fps_kernel()
    return _CACHE["nc"]


def run_on_cores(pos, **spmd_kwargs):
    """pos [16, 16384, 3] f32 -> (idx [16*2048] int32, BassKernelResults)."""
    pos = np.ascontiguousarray(np.asarray(pos, dtype=np.float32))
    assert pos.shape == (B, N, 3)
    nc = _get_nc()
    in_maps = [_make_inputs(pos[N_CLOUDS * c: N_CLOUDS * (c + 1)]) for c in range(N_CORES)]
    res = run_bass_kernel_spmd(nc, in_maps, core_ids=list(range(N_CORES)), **spmd_kwargs)
    idx = np.empty((B, S), np.int32)
    for core in range(N_CORES):
        outrow = res.results[core]["outrow"]  # [n_clouds, 3S]
        for c in range(N_CLOUDS):
            b = N_CLOUDS * core + c
            coords = np.ascontiguousarray(-outrow[c].reshape(S, 3))
            pb = np.ascontiguousarray(pos[b])
            lut = {pb[n].tobytes(): n for n in range(pb.shape[0])}
            loc = np.empty(S, np.int32)
            loc[0] = 0
            for si in range(1, S):
                loc[si] = lut.get(coords[si].tobytes(), -1)
            idx[b] = loc + b * N
    return idx.reshape(-1), res


def kernel(pos):
    idx, _ = run_on_cores(pos)
    return idx

